# revision 15
# baseline (speedup 1.0000x reference)
"""STFT (DFT-as-conv) kernel for Trainium2, 8 NeuronCores.

Problem: x (16, 262144) f32, hann-windowed DFT kernels wsin/wcos
(2048, 1, 2048); reference reflect-pads by 1024, convolves with hop 512
-> returns (real, -imag), each (16, 2048, 513) f32.

Strategy (two symmetry folds on top of an im2col matmul):
  - Data-parallel over batch: 2 batches per core.
  - Hop-block im2col: n_fft = 4*hop, so frame matrices are shifted
    views of block-transposed copies of the padded signal.
  - Time-reversal fold: z = y[n] +/- y[2048-n] halves contraction to
    1024; win[0] = 0 frees the n=0 lane for the cos n=1024 column.
  - Bin-parity fold: even/odd contraction lanes give E/O partial sums;
    host assembles out[k] = E+O, out[1024-k] = +/-(E-O); bin 512 row
    and frame 512 column are host matvecs; bins 1025+ are mirrors.
  - bf16 off-chip everywhere (PSUM accumulates f32): DMA transfers
    serialize globally in the hw model, so bytes are the bound.
  - Device does frames 0..511 only -> each unit is a single 512-wide
    PSUM group (136 matmuls total; PE sequencer cost is per
    instruction, so few long matmuls beat many short ones).
  - PE warmup matmuls during the DMA head so the p-state ramp
    (0.65 -> 1.2 -> 2.4 GHz after 3us continuous) is done before the
    first real matmul.
  - Few large DMAs (~630ns HWDGE each, exclusive); weights ride the
    ACT queue gated behind input arrivals so input transfers win the
    exclusive DMA engine early.
"""

import sys

sys.path.insert(0, "/opt/trn_rl_repo")

import numpy as np

BATCH = 16
LENGTH = 262144
N_FFT = 2048
HOP = 512
FRAMES = 513          # LENGTH // HOP + 1
DEV_F = 512           # frames computed on device; frame 512 on host
BT_COLS = 520         # block columns padded so shifted views stay in range
CORES = 8
B_PER_CORE = BATCH // CORES
N_UP = 8              # u' = kern*4 + mc, bins 0..511 in 4 chunks per kern
EXT = HOP * BT_COLS + 1537  # zero-extended xpad length for rev strides
N_WARM = 26           # PE p-state warmup matmuls (128 rows each)

_cache = {}


def _build_device_kernel(psbufs=3, n_warm=N_WARM, **_ignored):
    import concourse.bacc as bacc
    import concourse.mybir as mybir
    from concourse import tile

    nc = bacc.Bacc("TRN2", target_bir_lowering=False, debug=False,
                   num_devices=CORES)
    f32 = mybir.dt.float32
    bf16 = mybir.dt.bfloat16

    # xin[b, jj, src, e, m]: partition-major packed signal views:
    #   src 0: bte[jj,e,m] = xpad[512m + 256e + 2jj]
    #   src 1: rve[jj,e,m] = xpad[512m + 1536 - 256e - 2jj]
    #   src 2: bto[jj,e,m] = xpad[512m + 256e + 2jj + 1]
    #   src 3: rvo[jj,e,m] = xpad[512m + 1535 - 256e - 2jj]
    xin_d = nc.dram_tensor("xin", [B_PER_CORE, 128, 4, 2, BT_COLS], bf16,
                          kind="ExternalInput")
    # w[jj, u', par, c, mm]: folded parity weights for bins < 512
    w_d = nc.dram_tensor("w", [128, N_UP, 2, 4, 128], bf16,
                         kind="ExternalInput")
    # o[u', mm, b*1024 + half*512 + f]: half 0 = E, 1 = O
    o_d = nc.dram_tensor("o", [N_UP, 128, B_PER_CORE * 2 * DEV_F],
                         bf16, kind="ExternalOutput")

    with tile.TileContext(nc) as tc:
        with (
            tc.tile_pool(name="inp", bufs=1) as inp,
            tc.tile_pool(name="zp", bufs=1) as zpool,
            tc.tile_pool(name="wpool", bufs=1) as wpool,
            tc.tile_pool(name="op", bufs=8) as op,
            tc.tile_pool(name="psp", bufs=psbufs, space="PSUM") as psp,
            tc.tile_pool(name="wmp", bufs=1) as wmp,
            tc.tile_pool(name="wps", bufs=1, space="PSUM") as wps,
        ):
            ins = [None] * B_PER_CORE
            zt = [[[[None] * 4 for _ in range(B_PER_CORE)]
                   for _ in range(2)] for _ in range(2)]
            for b in range(B_PER_CORE):
                ins[b] = inp.tile([128, 4, 2, BT_COLS], bf16,
                                  name=f"in{b}", tag=f"in{b}")
                for par in range(2):
                    for s in range(2):
                        for c in range(4):
                            zt[par][s][b][c] = zpool.tile(
                                [128, DEV_F + 2], bf16,
                                name=f"z{par}{s}{b}{c}",
                                tag=f"z{par}{s}{b}{c}")
            wts = wpool.tile([128, N_UP, 2, 4, 128], bf16,
                             name="wt", tag="wt")
            scr = wmp.tile([128, 130], bf16, name="scr", tag="scr")
            gate = wmp.tile([2, 8], bf16, name="gate", tag="gate")

            # --- PE warmup: ramp the p-state while DMAs land ---
            nc.gpsimd.memset(scr, 0.0)
            wpsum = wps.tile([128, 128], f32, name="wpsum", tag="wpsum")
            for i in range(n_warm):
                nc.tensor.matmul(wpsum, scr[:, :128], scr[:, 2:130],
                                 start=True, stop=True)

            def fold(b, s, lo, hi, engs):
                # engs: list of engines cycled across the 8 (par, c) ops
                k = 0
                for par in range(2):
                    bt_t = ins[b][:, 2 * par]
                    rv_t = ins[b][:, 2 * par + 1]
                    for c in range(4):
                        eng = engs[k % len(engs)]
                        k += 1
                        sh, rh = c // 2, 1 - c // 2
                        o_ap = zt[par][s][b][c][:, lo:hi]
                        a_ap = bt_t[:, c % 2, lo + sh:hi + sh]
                        b_ap = rv_t[:, c % 2, lo + rh:hi + rh]
                        if eng is nc.scalar:   # ACT: add via activation
                            eng.add(o_ap, a_ap, b_ap)
                        else:
                            op_ = (eng.tensor_add, eng.tensor_sub)[s]
                            op_(out=o_ap, in0=a_ap, in1=b_ap)
                if s == 0:
                    # n=0 lane (c=0,jj=0) freed by win[0]=0: carry
                    # y_f[1024][m] = bte[0, 0, m+2] for the cos column.
                    nc.vector.tensor_copy(
                        out=zt[0][0][b][0][0:1, lo:hi],
                        in_=ins[b][0:1, 0, 0, lo + 2:hi + 2])

            # --- DMAs: inputs + outputs on SP; weights on ACT, each
            # weight chunk gated behind an input-consuming scratch copy
            # so inputs win the exclusive DMA engine in arrival order.
            SPLIT_A = 136
            SPLIT = 264
            # w0 E-side first (only piece the first matmul group needs)
            nc.scalar.dma_start(out=wts[:, 0, 0], in_=w_d[:, 0, 0])
            nc.sync.dma_start(out=ins[0][:, :, :, :SPLIT_A],
                              in_=xin_d[0][:, :, :, :SPLIT_A])
            nc.scalar.dma_start(out=wts[:, 0, 1], in_=w_d[:, 0, 1])
            nc.sync.dma_start(out=ins[0][:, :, :, SPLIT_A:SPLIT],
                              in_=xin_d[0][:, :, :, SPLIT_A:SPLIT])
            nc.sync.dma_start(out=ins[0][:, :, :, SPLIT:],
                              in_=xin_d[0][:, :, :, SPLIT:])
            # gate w1 on b0h1 arrival
            nc.scalar.copy(out=gate[0:1, 0:1], in_=ins[0][0:1, 0, 0, 0:1])
            nc.scalar.dma_start(out=wts[:, 1], in_=w_d[:, 1])
            nc.sync.dma_start(out=ins[1][:, :, :, :SPLIT],
                              in_=xin_d[1][:, :, :, :SPLIT])
            # gate w2:4 on b0h2 arrival
            nc.scalar.copy(out=gate[0:1, 1:2],
                           in_=ins[0][0:1, 0, 0, SPLIT:SPLIT + 1])
            nc.scalar.dma_start(out=wts[:, 2:4], in_=w_d[:, 2:4])
            nc.sync.dma_start(out=ins[1][:, :, :, SPLIT:],
                              in_=xin_d[1][:, :, :, SPLIT:])
            # gate w4:6 on b1h1, w6:8 on b1h2
            nc.scalar.copy(out=gate[0:1, 2:3], in_=ins[1][0:1, 0, 0, 0:1])
            nc.scalar.dma_start(out=wts[:, 4:6], in_=w_d[:, 4:6])
            nc.scalar.copy(out=gate[0:1, 3:4],
                           in_=ins[1][0:1, 0, 0, SPLIT:SPLIT + 1])
            nc.scalar.dma_start(out=wts[:, 6:8], in_=w_d[:, 6:8])

            # --- folds ---
            MIDA = 132
            MID = 258
            HI = DEV_F + 1   # z cols 0..513 (frame 511 reads z[.. +1])
            fold(0, 0, 0, MIDA, [nc.vector])
            fold(0, 0, MIDA, MID, [nc.vector])
            fold(0, 0, MID, HI, [nc.vector])
            fold(0, 1, 0, HI, [nc.gpsimd])      # z- b0 on Pool
            fold(1, 1, 0, HI, [nc.gpsimd])      # z- b1 on Pool

            # --- units ---
            sched = [(up, b) for b in range(B_PER_CORE)
                     for up in range(N_UP)]
            otj_map = {}
            b1_folded = False
            for idx, (up, b) in enumerate(sched):
                kern = up // 4
                if up not in otj_map:
                    otj_map[up] = op.tile(
                        [128, B_PER_CORE * 2 * DEV_F], bf16,
                        name="otj", tag="ot")
                ot = otj_map[up][:, b * 2 * DEV_F:(b + 1) * 2 * DEV_F]
                last = idx == len(sched) - 1
                if idx == 0:
                    groups = ((0, 130), (130, 128), (258, DEV_F - 258))
                elif last:
                    # tail: 2 groups + quartered [E1 O1 E2 O2] layout so
                    # the final copy/DMA chain overlaps the final matmuls
                    groups = ((0, 256), (256, 256))
                else:
                    groups = ((0, DEV_F),)
                for f0, ng in groups:
                    psE = psp.tile([128, ng], f32, name="psE", tag="psE")
                    psO = psp.tile([128, ng], f32, name="psO", tag="psO")
                    for c in range(4):
                        nc.tensor.matmul(
                            psE, wts[:, up, 0, c, :],
                            zt[0][kern][b][c][:, f0:f0 + ng],
                            start=(c == 0), stop=(c == 3))
                    for c in range(4):
                        nc.tensor.matmul(
                            psO, wts[:, up, 1, c, :],
                            zt[1][kern][b][c][:, f0:f0 + ng],
                            start=(c == 0), stop=(c == 3))
                    if last:
                        nc.vector.tensor_copy(
                            out=ot[:, 2 * f0:2 * f0 + ng], in_=psE)
                        nc.scalar.copy(
                            out=ot[:, 2 * f0 + ng:2 * f0 + 2 * ng],
                            in_=psO)
                        base = b * 2 * DEV_F
                        nc.sync.dma_start(
                            out=o_d[up, :, base + 2 * f0:
                                    base + 2 * f0 + 2 * ng],
                            in_=ot[:, 2 * f0:2 * f0 + 2 * ng])
                    else:
                        nc.vector.tensor_copy(
                            out=ot[:, f0:f0 + ng], in_=psE)
                        nc.scalar.copy(
                            out=ot[:, DEV_F + f0:DEV_F + f0 + ng],
                            in_=psO)
                if not last:
                    base = b * 2 * DEV_F
                    nc.sync.dma_start(
                        out=o_d[up, :, base:base + 2 * DEV_F],
                        in_=ot[:, :2 * DEV_F] if b == 0 else ot)
                if idx == 1 and not b1_folded:
                    # b1 z+ folds emitted here: data has landed by now,
                    # so they never block queue-heads ahead of copies
                    b1_folded = True
                    fold(1, 0, 0, HI, [nc.vector])
    nc.compile()
    return nc


def _get_nc():
    if "nc" not in _cache:
        _cache["nc"] = _build_device_kernel()
    return _cache["nc"]


def _host_prep(x, wsin, wcos):
    import ml_dtypes
    from numpy.lib.stride_tricks import as_strided

    x = np.asarray(x, dtype=np.float32)
    wsin = np.asarray(wsin, dtype=np.float32).reshape(N_FFT, N_FFT)
    wcos = np.asarray(wcos, dtype=np.float32).reshape(N_FFT, N_FFT)

    xpad = np.pad(x, ((0, 0), (N_FFT // 2, N_FFT // 2)), mode="reflect")
    xe = np.zeros((BATCH, EXT), np.float32)
    xe[:, :xpad.shape[1]] = xpad
    sb = xe.strides[1]
    s0 = xe.strides[0]

    xin = np.empty((BATCH, 128, 4, 2, BT_COLS), np.float32)
    shape = (BATCH, 128, 2, BT_COLS)
    xin[:, :, 0] = as_strided(xe, shape, (s0, 2 * sb, 256 * sb, 512 * sb))
    xin[:, :, 2] = as_strided(xe[:, 1:], shape,
                              (s0, 2 * sb, 256 * sb, 512 * sb))
    xin[:, :, 1] = as_strided(xe[:, 1536:], shape,
                              (s0, -2 * sb, -256 * sb, 512 * sb))
    xin[:, :, 3] = as_strided(xe[:, 1535:], shape,
                              (s0, -2 * sb, -256 * sb, 512 * sb))
    xin = xin.astype(ml_dtypes.bfloat16)

    # folded parity weights for bin rows k < 512: wf[jj, u', par, c, mm]
    wf = np.empty((128, N_UP, 2, 4, 128), np.float32)
    jj = np.arange(128)
    for kern, wm in enumerate((wcos, -wsin)):
        for mc in range(4):
            rows = wm[128 * mc:128 * mc + 128]       # (128 bins, 2048)
            for c in range(4):
                n_ev = 256 * c + 2 * jj
                wf[:, kern * 4 + mc, 0, c, :] = rows[:, n_ev].T
                wf[:, kern * 4 + mc, 1, c, :] = rows[:, n_ev + 1].T
    # n=0 even lane is dead (win[0] = 0): carry the cos n=1024 column
    wf[0, 0:4, 0, 0, :] = wcos[:512, 1024].reshape(4, 128)
    wf = wf.astype(ml_dtypes.bfloat16)

    # host bin-512 row (frames 0..512) and frame-512 column (all bins)
    fr = np.lib.stride_tricks.sliding_window_view(
        xpad, N_FFT, axis=1)[:, ::HOP]               # (B, 513, 2048)
    row512 = np.empty((2, BATCH, FRAMES), np.float32)
    for kern, wm in enumerate((wcos, -wsin)):
        row512[kern] = np.einsum('bfn,n->bf', fr, wm[512],
                                 optimize=True).astype(np.float32)
    y512 = np.ascontiguousarray(fr[:, 512])          # (B, 2048)
    col512 = np.empty((2, BATCH, N_FFT), np.float32)
    col512[0] = y512 @ wcos.T
    col512[1] = y512 @ (-wsin).T
    return xin, wf, row512, col512


def _host_assemble(outs, row512, col512):
    # outs: 8 arrays (8, 128, 2*2*512) bf16; E/O halves per batch,
    # except (up=7, b=1) which is quartered [E1 O1 E2 O2] (tail split)
    outs = [np.asarray(o, np.float32) for o in outs]
    per_batch_E, per_batch_O = [], []
    for o in outs:
        for b in range(B_PER_CORE):
            base = b * 2 * DEV_F
            per_batch_E.append(o[:, :, base:base + DEV_F].copy())
            per_batch_O.append(
                o[:, :, base + DEV_F:base + 2 * DEV_F].copy())
            if b == 1:
                q = o[7, :, base:base + 2 * DEV_F]
                per_batch_E[-1][7] = np.concatenate(
                    [q[:, 0:256], q[:, 512:768]], axis=1)
                per_batch_O[-1][7] = np.concatenate(
                    [q[:, 256:512], q[:, 768:1024]], axis=1)
    E = np.stack(per_batch_E).reshape(BATCH, 2, 512, DEV_F)
    O = np.stack(per_batch_O).reshape(BATCH, 2, 512, DEV_F)

    outs_full = []
    for kern, msign in ((0, 1.0), (1, -1.0)):
        lo = E[:, kern] + O[:, kern]               # bins 0..511
        hi = E[:, kern] - O[:, kern]               # bins 1024-k
        if kern == 1:
            hi = -hi
        head = np.concatenate(
            [lo, row512[kern][:, None, :DEV_F], hi[:, 511:0:-1],
             hi[:, 0:1]], axis=1)                   # bins 0..1024
        full = np.concatenate([head, msign * head[:, 1023:0:-1]], axis=1)
        full = np.concatenate(
            [full, col512[kern][:, :, None]], axis=2)  # frame 512
        outs_full.append(np.ascontiguousarray(full, dtype=np.float32))
    return tuple(outs_full)


def kernel(x, wsin, wcos):
    from concourse.bass_utils import run_bass_kernel_spmd

    nc = _get_nc()
    xin, wf, row512, col512 = _host_prep(x, wsin, wcos)
    in_maps = [
        {"xin": xin[i * B_PER_CORE:(i + 1) * B_PER_CORE], "w": wf}
        for i in range(CORES)
    ]
    res = run_bass_kernel_spmd(nc, in_maps, core_ids=list(range(CORES)))
    return _host_assemble(
        [res.results[i]["o"] for i in range(CORES)], row512, col512)
